# revision 38
# baseline (speedup 1.0000x reference)
"""Multi-head attention (softmax over the QUERY axis) on 8 TRN2 NeuronCores.

Sharding: 2 batches x 4 head-groups (4 heads each) -> 8 cores.
Each core computes, for its (batch b, heads 4g..4g+3):
    qkT = W_{q,k} @ x_b^T + b_{q,k}   [512, 2048]   (e_out on partitions)
    V   = x_b @ W_v^T + b_v           [2048, 256]
    S'  = K Q^T (scores TRANSPOSED)   [k, q] per head
    P   = exp(S'/8) with fused row-sum -> denom[k]  (softmax over q == free dim)
    outT= sum_k (V[k,:]/denom[k]) P[k,:]            [d, q] per head
    part= outT^T @ WoT_g              [2048, 1024]  (partial per head pair)
Host sums the partials per batch (bf16) and adds bo.

The ACT exp stream (~165us/core) is the critical path; everything else
is scheduled around keeping ACT saturated and the PE clock warm:
- attn.V matmuls (M=64) use disjoint PSUM partition quadrants per head
  (PE col tiling) and are emitted hh-adjacent so pairs co-execute.
- Per k-tile: [scores half0][exp x2][attn.V prev group + fillers]
  [scores half1][exp x2][fillers][den -> vs]. Fillers (qkT/V projection
  groups, pair0's tail, output projection groups) follow an explicit
  per-kt schedule that keeps PE gaps small in BOTH pair loops -- idle
  gaps >~2us drop the PE to its mid p-state (2x slower) which then
  delays score refills and starves ACT.
- Input DMA is spread over three engine queues (descriptor issue on one
  queue serializes at ~0.6us each).
- The tail projects q-half 0 while the final exps drain, using the
  freed score-PSUM tiles and copies alternating DVE/ACT.
"""

import sys

if "/opt/trn_rl_repo" not in sys.path:
    sys.path.insert(0, "/opt/trn_rl_repo")

import numpy as np
import ml_dtypes

import concourse.bass as bass
import concourse.mybir as mybir
import concourse.tile as tile
from concourse import bacc
from concourse.bass_utils import run_bass_kernel_spmd

F32 = mybir.dt.float32
BF16 = mybir.dt.bfloat16
AF = mybir.ActivationFunctionType

B, S, E, H = 2, 2048, 1024, 16
HL = 4  # heads per core
DH = 64
QK = 512  # q+k out dims per core (2*HL*DH)
V3 = 768  # q+k+v out dims per core
NCORES = 8

ET = E // 128  # 8 e-tiles
ST = S // 128  # 16 s-tiles
SC = S // 512  # 4 s/q chunks of 512
KT = ST  # 16 k-tiles
FG = 4  # k-tiles per attn.V accumulation group

LAST_RESULTS = None


def build_kernel():
    nc = bacc.Bacc("TRN2", target_bir_lowering=False, debug=False, num_devices=NCORES)

    # inputs are packed host-side in partition-major et-blocked layout so
    # each loads with ONE contiguous-per-partition DMA (issue cost dominates
    # DMA throughput: ~2us per issue regardless of size)
    xTa = nc.dram_tensor("xTa", [128, ET * 1024], BF16, kind="ExternalInput")
    xTb = nc.dram_tensor("xTb", [128, ET * 1024], BF16, kind="ExternalInput")
    wTq = nc.dram_tensor("wTq", [128, ET * 256], BF16, kind="ExternalInput")
    wTk = nc.dram_tensor("wTk", [128, ET * 256], BF16, kind="ExternalInput")
    wTv = nc.dram_tensor("wTv", [128, ET * 256], BF16, kind="ExternalInput")
    bq = nc.dram_tensor("bq", [128, 4], F32, kind="ExternalInput")
    bv = nc.dram_tensor("bv", [1, 256], BF16, kind="ExternalInput")
    woT = nc.dram_tensor("woT", [2 * 128, E], BF16, kind="ExternalInput")
    out0 = nc.dram_tensor("out0", [S, E], BF16, kind="ExternalOutput")
    out1 = nc.dram_tensor("out1", [S, E], BF16, kind="ExternalOutput")

    with tile.TileContext(nc) as tc:
        with (
            tc.tile_pool(name="persist", bufs=1) as persist,
            tc.tile_pool(name="smalls", bufs=6) as smalls,
            tc.tile_pool(name="expp", bufs=2 * FG + 1) as expp,
            tc.tile_pool(name="vsp", bufs=2 * FG + 2) as vsp,
            tc.tile_pool(name="fout", bufs=3) as foutp,
            tc.tile_pool(name="mm_ps", bufs=2, space="PSUM") as mm_ps,
            tc.tile_pool(name="sp_ps", bufs=2, space="PSUM") as sp_ps,
            tc.tile_pool(name="ot_ps", bufs=1, space="PSUM") as ot_ps,
        ):
            qk_sb = persist.tile([128, 4, S], BF16, tag="qk")
            v_sb = persist.tile([128, ST, 256], F32, tag="v")
            outT_f32 = persist.tile([128, 2, S], F32, tag="outT")
            outT_bf = persist.tile([128, 2, S], BF16, tag="outT_bf")
            bq_sb = persist.tile([128, 4], F32, tag="bq")
            bv_sb = persist.tile([1, 256], BF16, tag="bv")
            ones_sb = persist.tile([1, 512], BF16, tag="ones")
            xt_sb = persist.tile([128, ET, S], BF16, tag="xt")
            wt_sb = persist.tile([128, ET, V3], BF16, tag="wt")
            wo_sb = persist.tile([128, 2, E], BF16, tag="wo")

            nc.vector.memset(ones_sb[:], 1.0)
            # the ACT queue must carry NO dma issues (they would delay every
            # exp); big contiguous DMAs in strict priority order on one queue
            # (in-flow is bandwidth-bound; a second queue would steal engines
            # from the critical prefix)
            nc.sync.dma_start(wt_sb[:, :, 0:256], wTq[:])
            nc.sync.dma_start(xt_sb[:, :, 0:1024], xTa[:])
            nc.sync.dma_start(wt_sb[:, :, 256:QK], wTk[:])
            nc.sync.dma_start(wt_sb[:, :, QK:V3], wTv[:])
            nc.sync.dma_start(xt_sb[:, :, 1024:2048], xTb[:])
            nc.gpsimd.dma_start(bq_sb[:], bq[:])
            nc.gpsimd.dma_start(bv_sb[:], bv[:])
            for p in range(2):
                nc.gpsimd.dma_start(wo_sb[:, p, :], woT[p * 128 : (p + 1) * 128, :])

            # ---- emitters for qkT / V accumulation groups ----------------
            def emit_qk_group(eo, sc):
                pt = mm_ps.tile([128, 512], F32, tag="mmps")
                for et in range(ET):
                    nc.tensor.matmul(
                        pt[:],
                        wt_sb[:, et, eo * 128 : (eo + 1) * 128],
                        xt_sb[:, et, sc * 512 : (sc + 1) * 512],
                        start=(et == 0),
                        stop=(et == ET - 1),
                    )
                nc.vector.tensor_scalar_add(
                    qk_sb[:, eo, sc * 512 : (sc + 1) * 512],
                    in0=pt[:],
                    scalar1=bq_sb[:, eo : eo + 1],
                )

            def emit_v_group(st, ph):
                # V projection for one head pair (128 of the 256 v-dims);
                # pair 1's half runs inside the pair-1 loop as PE filler
                pt = mm_ps.tile([128, 512], F32, tag="mmps")
                c0 = QK + ph * 128
                for et in range(ET):
                    nc.tensor.matmul(
                        pt[:, :128],
                        xt_sb[:, et, st * 128 : (st + 1) * 128],
                        wt_sb[:, et, c0 : c0 + 128],
                        start=(et == 0),
                        stop=False,
                    )
                nc.tensor.matmul(  # + ones^T bv (bias row)
                    pt[:, :128],
                    ones_sb[0:1, 0:128],
                    bv_sb[0:1, ph * 128 : (ph + 1) * 128],
                    start=False,
                    stop=True,
                )
                nc.vector.tensor_copy(
                    v_sb[:, st, ph * 128 : (ph + 1) * 128], pt[:, :128]
                )

            def emit_d_group(p, st, out_dram):
                # in-loop variant: mm_ps tiles + DVE copies; output DMAs
                # alternate SP/gpsimd queues so neither backs up
                ot = foutp.tile([128, E], BF16, tag="fout", name=f"fo_{p}_{st}")
                for nck in range(2):
                    pt = mm_ps.tile([128, 512], F32, tag="mmps", name=f"fp_{p}_{st}_{nck}")
                    nc.tensor.matmul(
                        pt[:],
                        outT_bf[:, p, st * 128 : (st + 1) * 128],
                        wo_sb[:, p, nck * 512 : (nck + 1) * 512],
                        start=True,
                        stop=True,
                    )
                    nc.vector.tensor_copy(ot[:, nck * 512 : (nck + 1) * 512], pt[:])
                dq = nc.sync if st % 2 == 0 else nc.gpsimd
                dq.dma_start(out_dram[st * 128 : (st + 1) * 128, :], ot[:])

            def emit_d_group_sp(p, st, out_dram):
                # tail variant: freed score-PSUM tile, one copy, DVE/ACT alternating
                ot = foutp.tile([128, E], BF16, tag="fout", name=f"fo_{p}_{st}")
                pt = sp_ps.tile([128, 1024], F32, tag="sp", name=f"fsp_{p}_{st}")
                for nck in range(2):
                    nc.tensor.matmul(
                        pt[:, nck * 512 : (nck + 1) * 512],
                        outT_bf[:, p, st * 128 : (st + 1) * 128],
                        wo_sb[:, p, nck * 512 : (nck + 1) * 512],
                        start=True,
                        stop=True,
                    )
                if st % 2 == 0:
                    nc.vector.tensor_copy(ot[:], pt[:])
                else:
                    nc.scalar.copy(ot[:], pt[:])
                dq = nc.sync if st % 2 == 0 else nc.gpsimd
                dq.dma_start(out_dram[st * 128 : (st + 1) * 128, :], ot[:])

            # ---- attn.V slices: col-tiled hh pairs -----------------------
            c_state = {}

            def emit_c_slices(p, g, half, jpair, exs, vss):
                if jpair == 0:
                    c_state[half] = ot_ps.tile(
                        [128, 1024], F32, tag="otps", name=f"oTt_{p}_{g}_{half}"
                    )
                oTt = c_state[half]
                for j in (2 * jpair, 2 * jpair + 1):
                    kt = FG * g + j
                    for qc in range(2):
                        q0 = half * 1024 + qc * 512
                        for hh in range(2):
                            nc.tensor.matmul(
                                oTt[
                                    hh * 64 : (hh + 1) * 64,
                                    qc * 512 : (qc + 1) * 512,
                                ],
                                vss[kt][:, hh, :],
                                exs[kt][:, hh, q0 : q0 + 512],
                                start=(j == 0),
                                stop=(j == FG - 1),
                            )
                if jpair == 1:
                    dst = outT_f32[:, p, half * 1024 : (half + 1) * 1024]
                    if g == 0:
                        nc.vector.tensor_copy(dst, oTt[:])
                    else:
                        nc.vector.tensor_add(dst, dst, oTt[:])

            def emit_cast(p, half):
                nc.vector.tensor_copy(
                    outT_bf[:, p, half * 1024 : (half + 1) * 1024],
                    outT_f32[:, p, half * 1024 : (half + 1) * 1024],
                )

            # ---- pre-attention ------------------------------------------
            # warmup matmuls (K=1 on the memset ones row) ramp the PE clock
            # while the input DMAs stream in
            wpt = mm_ps.tile([128, 512], F32, tag="mmps", name="warm")
            for r in range(18):
                nc.tensor.matmul(
                    wpt[:],
                    ones_sb[0:1, 0:128],
                    ones_sb[0:1, 0:512],
                    start=True,
                    stop=True,
                )
            emit_qk_group(0, 0)  # Q pair0 cols 0-511
            emit_qk_group(0, 1)
            emit_qk_group(2, 0)  # K pair0 cols 0-511 (kts 0-3)

            def qg(eo, sc):
                return lambda: emit_qk_group(eo, sc)

            def vg(st, ph):
                return lambda: emit_v_group(st, ph)

            def dg(p, st, out_dram):
                return lambda: emit_d_group(p, st, out_dram)

            def cs(p, g, half, jpair, exs, vss):
                return lambda: emit_c_slices(p, g, half, jpair, exs, vss)

            def ct(p, half):
                return lambda: emit_cast(p, half)

            # ---- attention per head pair ---------------------------------
            # Each pair walks a (kt, half) schedule; afterwork[pos] is the
            # PE work emitted right after that position's score matmuls+exps
            # (the previous group's attn.V slices + projection fillers).
            # Pair 0 defers k-tiles 0-4's second q-half so the exp stream
            # starts on sc0/sc1 input data while sc2/sc3 still stream in.
            def CS(p, kc, exs, vss):
                return lambda: emit_c_slices(
                    p, kc // FG - 1, (kc % FG) // 2, kc % 2, exs, vss
                )

            p0_exs = None
            p0_vss = None
            for p in range(2):
                exs = {}
                vss = {}
                dens = {}
                if p == 0:
                    sched = [
                        (0, 0), (1, 0), (2, 0), (3, 0), (4, 0),
                        (0, 1), (1, 1), (2, 1), (3, 1), (4, 1),
                    ] + [(kt, h) for kt in range(5, KT) for h in (0, 1)]
                    aw = {
                        0: [vg(0, 0), vg(1, 0)],
                        1: [qg(2, 1), vg(2, 0)],
                        2: [vg(3, 0), qg(0, 2)],
                        3: [vg(4, 0), qg(0, 3)],
                        4: [qg(2, 2), vg(5, 0)],
                        5: [vg(6, 0)],
                        6: [qg(2, 3), vg(7, 0)],
                        7: [vg(8, 0)],
                        8: [vg(9, 0)],
                        9: [CS(0, 4, exs, vss), vg(10, 0)],
                        10: [CS(0, 5, exs, vss)],
                        11: [qg(1, 0), vg(11, 0)],
                        12: [CS(0, 6, exs, vss)],
                        13: [qg(1, 1), vg(12, 0)],
                        14: [CS(0, 7, exs, vss)],
                        15: [vg(13, 0)],
                        16: [CS(0, 8, exs, vss)],
                        17: [vg(14, 0)],
                        18: [CS(0, 9, exs, vss)],
                        19: [vg(15, 0)],
                        20: [CS(0, 10, exs, vss)],
                        21: [qg(3, 0), vg(8, 1)],
                        22: [CS(0, 11, exs, vss)],
                        23: [qg(3, 1), vg(9, 1)],
                        24: [CS(0, 12, exs, vss), vg(10, 1)],
                        25: [vg(11, 1)],
                        26: [CS(0, 13, exs, vss), vg(12, 1)],
                        27: [vg(13, 1)],
                        28: [CS(0, 14, exs, vss)],
                        29: [qg(1, 2)],
                        30: [CS(0, 15, exs, vss)],
                    }
                else:
                    sched = [(kt, h) for kt in range(KT) for h in (0, 1)]
                    aw = {
                        0: [qg(1, 3)],
                        1: [vg(0, 1), vg(1, 1)],
                        2: [cs(0, 3, 0, 0, p0_exs, p0_vss)],
                        3: [cs(0, 3, 0, 1, p0_exs, p0_vss), vg(2, 1)],
                        4: [ct(0, 0)],
                        5: [cs(0, 3, 1, 0, p0_exs, p0_vss), vg(3, 1)],
                        6: [cs(0, 3, 1, 1, p0_exs, p0_vss)],
                        7: [ct(0, 1), vg(4, 1)],
                        8: [CS(1, 4, exs, vss), dg(0, 0, out0)],
                        9: [dg(0, 1, out0), vg(5, 1)],
                        10: [CS(1, 5, exs, vss), qg(3, 2)],
                        11: [dg(0, 2, out0), vg(6, 1)],
                        12: [CS(1, 6, exs, vss), dg(0, 3, out0)],
                        13: [dg(0, 4, out0), vg(7, 1)],
                        14: [CS(1, 7, exs, vss), dg(0, 5, out0)],
                        15: [dg(0, 6, out0), dg(0, 7, out0)],
                        16: [CS(1, 8, exs, vss), qg(3, 3)],
                        17: [dg(0, 8, out0)],
                        18: [CS(1, 9, exs, vss), dg(0, 9, out0)],
                        19: [dg(0, 10, out0)],
                        20: [CS(1, 10, exs, vss), dg(0, 11, out0)],
                        21: [dg(0, 12, out0), dg(0, 13, out0)],
                        22: [CS(1, 11, exs, vss)],
                        23: [dg(0, 14, out0)],
                        24: [CS(1, 12, exs, vss), dg(0, 15, out0)],
                        26: [CS(1, 13, exs, vss)],
                        27: [vg(14, 1)],
                        28: [CS(1, 14, exs, vss)],
                        29: [vg(15, 1)],
                        30: [CS(1, 15, exs, vss), cs(1, 3, 0, 0, exs, vss)],
                    }
                for pos, (kt, half) in enumerate(sched):
                    if kt not in exs:
                        exs[kt] = expp.tile(
                            [128, 2, S], BF16, tag="exp", name=f"ex_{p}_{kt}"
                        )
                        dens[kt] = smalls.tile(
                            [128, 2, 2], F32, tag="den", name=f"den_{p}_{kt}"
                        )
                    ex = exs[kt]
                    den = dens[kt]
                    sps = {}
                    for hh in range(2):
                        sps[hh] = sp_ps.tile(
                            [128, 1024], F32, tag="sp", name=f"sp_{p}_{kt}_{half}_{hh}"
                        )
                    for qc in range(2):
                        q0 = half * 1024 + qc * 512
                        for hh in range(2):
                            nc.tensor.matmul(
                                sps[hh][:, qc * 512 : (qc + 1) * 512],
                                qk_sb[
                                    hh * 64 : (hh + 1) * 64,
                                    2 + p,
                                    kt * 128 : (kt + 1) * 128,
                                ],
                                qk_sb[hh * 64 : (hh + 1) * 64, p, q0 : q0 + 512],
                                start=True,
                                stop=True,
                            )
                    for hh in range(2):
                        nc.scalar.activation(
                            ex[:, hh, half * 1024 : (half + 1) * 1024],
                            sps[hh][:],
                            AF.Exp,
                            scale=0.125,
                            accum_out=den[:, hh, half : half + 1],
                        )
                    for w in aw.get(pos, []):
                        w()
                    if half == 1:
                        # den -> vs chain on gpsimd (+DVE reciprocal): keeps
                        # the critical vs path off the bursty DVE queue
                        dsum = smalls.tile([128, 2], F32, tag="dsum")
                        nc.gpsimd.tensor_add(dsum[:], den[:, :, 0], den[:, :, 1])
                        rec = smalls.tile([128, 2], F32, tag="rec")
                        nc.vector.reciprocal(rec[:], dsum[:])
                        vs = vsp.tile([128, 2, DH], BF16, tag="vs")
                        vss[kt] = vs
                        for hh in range(2):
                            nc.gpsimd.tensor_scalar_mul(
                                vs[:, hh, :],
                                in0=v_sb[
                                    :, kt, (2 * p + hh) * 64 : (2 * p + hh + 1) * 64
                                ],
                                scalar1=rec[:, hh : hh + 1],
                            )
                if p == 0:
                    p0_exs = exs
                    p0_vss = vss

            # ---- tail: pair1 group3 attn.V + projection ------------------
            # half0's projection overlaps the final half1 exps
            emit_c_slices(1, 3, 0, 1, exs, vss)
            emit_cast(1, 0)
            for st in range(ST // 2):
                emit_d_group_sp(1, st, out1)
            emit_c_slices(1, 3, 1, 0, exs, vss)
            emit_c_slices(1, 3, 1, 1, exs, vss)
            emit_cast(1, 1)
            for st in range(ST // 2, ST):
                emit_d_group_sp(1, st, out1)

    nc.compile()
    return nc


def _shard_inputs(input, Wqkv, bqkv, Wo):
    """Build the 8 per-core input dicts (host-side layout/sharding)."""
    bf16 = ml_dtypes.bfloat16
    in_maps = []
    for c in range(NCORES):
        b = c // 4
        g = c % 4
        heads = range(4 * g, 4 * g + 4)
        rows = (
            [slice(64 * h, 64 * h + 64) for h in heads]
            + [slice(E + 64 * h, E + 64 * h + 64) for h in heads]
            + [slice(2 * E + 64 * h, 2 * E + 64 * h + 64) for h in heads]
        )
        W_sel = np.concatenate([Wqkv[s] for s in rows], axis=0)  # [768, 1024]
        b_sel = np.concatenate([bqkv[s] for s in rows], axis=0)  # [768]
        xf = input[b].T.reshape(ET, 128, S)  # [et, p, s]
        wf = W_sel.T.reshape(ET, 128, V3)  # [et, p, v]
        in_maps.append(
            {
                "xTa": np.ascontiguousarray(
                    xf[:, :, 0:1024].transpose(1, 0, 2).reshape(128, ET * 1024)
                ).astype(bf16),
                "xTb": np.ascontiguousarray(
                    xf[:, :, 1024:2048].transpose(1, 0, 2).reshape(128, ET * 1024)
                ).astype(bf16),
                "wTq": np.ascontiguousarray(
                    wf[:, :, 0:256].transpose(1, 0, 2).reshape(128, ET * 256)
                ).astype(bf16),
                "wTk": np.ascontiguousarray(
                    wf[:, :, 256:QK].transpose(1, 0, 2).reshape(128, ET * 256)
                ).astype(bf16),
                "wTv": np.ascontiguousarray(
                    wf[:, :, QK:V3].transpose(1, 0, 2).reshape(128, ET * 256)
                ).astype(bf16),
                "bq": np.ascontiguousarray(b_sel[:QK].reshape(4, 128).T),
                "bv": np.ascontiguousarray(b_sel[QK:V3].reshape(1, 256)).astype(bf16),
                "woT": np.ascontiguousarray(
                    Wo[:, 4 * g * DH : 4 * (g + 1) * DH].T
                ).astype(bf16),
            }
        )
    return in_maps


def kernel(input, Wqkv, bqkv, Wo, bo, _trace=False):
    global LAST_RESULTS
    input = np.asarray(input, dtype=np.float32)
    Wqkv = np.asarray(Wqkv, dtype=np.float32)
    bqkv = np.asarray(bqkv, dtype=np.float32)
    Wo = np.asarray(Wo, dtype=np.float32)
    bo = np.asarray(bo, dtype=np.float32)

    nc = build_kernel()
    in_maps = _shard_inputs(input, Wqkv, bqkv, Wo)
    kwargs = {}
    if _trace:
        kwargs = dict(trace=True, trace_cores=[0])
    res = run_bass_kernel_spmd(nc, in_maps, core_ids=list(range(NCORES)), **kwargs)
    LAST_RESULTS = res

    out = np.zeros((B, S, E), dtype=np.float32)
    for c in range(NCORES):
        out[c // 4] += res.results[c]["out0"].astype(np.float32)
        out[c // 4] += res.results[c]["out1"].astype(np.float32)
    out += bo
    return out


# revision 39
# speedup vs baseline: 1.2823x; 1.2823x over previous
"""Multi-head attention (softmax over the QUERY axis) on 8 TRN2 NeuronCores.

Sharding: 2 batches x 4 head-groups (4 heads each) -> 8 cores.
Each core computes, for its (batch b, heads 4g..4g+3):
    qkT = W_{q,k} @ x_b^T + b_{q,k}   [512, 2048]   (e_out on partitions)
    V   = x_b @ W_v^T + b_v           [2048, 256]
    S'  = K Q^T (scores TRANSPOSED)   [k, q] per head
    P   = exp(S'/8) with fused row-sum -> denom[k]  (softmax over q == free dim)
    outT= sum_k (V[k,:]/denom[k]) P[k,:]            [d, q] per head
    part= outT^T @ WoT_g              [2048, 1024]  (partial per head pair)
Host sums the partials per batch (bf16) and adds bo.

The ACT exp stream (~165us/core) is the critical path; everything else
is scheduled around keeping ACT saturated and the PE clock warm:
- attn.V matmuls (M=64) use disjoint PSUM partition quadrants per head
  (PE col tiling) and are emitted hh-adjacent so pairs co-execute.
- Per k-tile: [scores half0][exp x2][attn.V prev group + fillers]
  [scores half1][exp x2][fillers][den -> vs]. Fillers (qkT/V projection
  groups, pair0's tail, output projection groups) follow an explicit
  per-kt schedule that keeps PE gaps small in BOTH pair loops -- idle
  gaps >~2us drop the PE to its mid p-state (2x slower) which then
  delays score refills and starves ACT.
- Input DMA is spread over three engine queues (descriptor issue on one
  queue serializes at ~0.6us each).
- The tail projects q-half 0 while the final exps drain, using the
  freed score-PSUM tiles and copies alternating DVE/ACT.
"""

import sys

if "/opt/trn_rl_repo" not in sys.path:
    sys.path.insert(0, "/opt/trn_rl_repo")

import numpy as np
import ml_dtypes

import concourse.bass as bass
import concourse.mybir as mybir
import concourse.tile as tile
from concourse import bacc
from concourse.bass_utils import run_bass_kernel_spmd

F32 = mybir.dt.float32
BF16 = mybir.dt.bfloat16
AF = mybir.ActivationFunctionType

B, S, E, H = 2, 2048, 1024, 16
HL = 4  # heads per core
DH = 64
QK = 512  # q+k out dims per core (2*HL*DH)
V3 = 768  # q+k+v out dims per core
NCORES = 8

ET = E // 128  # 8 e-tiles
ST = S // 128  # 16 s-tiles
SC = S // 512  # 4 s/q chunks of 512
KT = ST  # 16 k-tiles
FG = 4  # k-tiles per attn.V accumulation group

LAST_RESULTS = None


def build_kernel():
    nc = bacc.Bacc("TRN2", target_bir_lowering=False, debug=False, num_devices=NCORES)

    # inputs are packed host-side in partition-major et-blocked layout so
    # each loads with ONE contiguous-per-partition DMA (issue cost dominates
    # DMA throughput: ~2us per issue regardless of size)
    xTa = nc.dram_tensor("xTa", [128, ET * 1024], BF16, kind="ExternalInput")
    xTb = nc.dram_tensor("xTb", [128, ET * 1024], BF16, kind="ExternalInput")
    wTq = nc.dram_tensor("wTq", [128, ET * 256], BF16, kind="ExternalInput")
    wTk = nc.dram_tensor("wTk", [128, ET * 256], BF16, kind="ExternalInput")
    wTv = nc.dram_tensor("wTv", [128, ET * 256], BF16, kind="ExternalInput")
    bq = nc.dram_tensor("bq", [128, 4], F32, kind="ExternalInput")
    bv = nc.dram_tensor("bv", [1, 256], BF16, kind="ExternalInput")
    woT = nc.dram_tensor("woT", [2 * 128, E], BF16, kind="ExternalInput")
    out0 = nc.dram_tensor("out0", [S, E], BF16, kind="ExternalOutput")
    out1 = nc.dram_tensor("out1", [S, E], BF16, kind="ExternalOutput")

    with tile.TileContext(nc) as tc:
        with (
            tc.tile_pool(name="persist", bufs=1) as persist,
            tc.tile_pool(name="smalls", bufs=6) as smalls,
            tc.tile_pool(name="expp", bufs=2 * FG + 1) as expp,
            tc.tile_pool(name="vsp", bufs=2 * FG + 2) as vsp,
            tc.tile_pool(name="fout", bufs=3) as foutp,
            tc.tile_pool(name="mm_ps", bufs=2, space="PSUM") as mm_ps,
            tc.tile_pool(name="sp_ps", bufs=2, space="PSUM") as sp_ps,
            tc.tile_pool(name="ot_ps", bufs=1, space="PSUM") as ot_ps,
        ):
            qk_sb = persist.tile([128, 4, S], BF16, tag="qk")
            v_sb = persist.tile([128, ST, 256], F32, tag="v")
            outT_f32 = persist.tile([128, 2, S], F32, tag="outT")
            outT_bf = persist.tile([128, 2, S], BF16, tag="outT_bf")
            bq_sb = persist.tile([128, 4], F32, tag="bq")
            bv_sb = persist.tile([1, 256], BF16, tag="bv")
            ones_sb = persist.tile([1, 512], BF16, tag="ones")
            xt_sb = persist.tile([128, ET, S], BF16, tag="xt")
            wt_sb = persist.tile([128, ET, V3], BF16, tag="wt")
            wo_sb = persist.tile([128, 2, E], BF16, tag="wo")

            nc.vector.memset(ones_sb[:], 1.0)
            # the ACT queue must carry NO dma issues (they would delay every
            # exp); big contiguous DMAs in strict priority order on one queue
            # (in-flow is bandwidth-bound; a second queue would steal engines
            # from the critical prefix)
            nc.sync.dma_start(wt_sb[:, :, 0:256], wTq[:])
            nc.sync.dma_start(xt_sb[:, :, 0:1024], xTa[:])
            nc.sync.dma_start(wt_sb[:, :, 256:QK], wTk[:])
            nc.sync.dma_start(wt_sb[:, :, QK:V3], wTv[:])
            nc.sync.dma_start(xt_sb[:, :, 1024:2048], xTb[:])
            nc.gpsimd.dma_start(bq_sb[:], bq[:])
            nc.gpsimd.dma_start(bv_sb[:], bv[:])
            for p in range(2):
                nc.gpsimd.dma_start(wo_sb[:, p, :], woT[p * 128 : (p + 1) * 128, :])

            # ---- emitters for qkT / V accumulation groups ----------------
            def emit_qk_group(eo, sc):
                pt = mm_ps.tile([128, 512], F32, tag="mmps")
                for et in range(ET):
                    nc.tensor.matmul(
                        pt[:],
                        wt_sb[:, et, eo * 128 : (eo + 1) * 128],
                        xt_sb[:, et, sc * 512 : (sc + 1) * 512],
                        start=(et == 0),
                        stop=(et == ET - 1),
                    )
                nc.vector.tensor_scalar_add(
                    qk_sb[:, eo, sc * 512 : (sc + 1) * 512],
                    in0=pt[:],
                    scalar1=bq_sb[:, eo : eo + 1],
                )

            def emit_v_group(st, ph):
                # V projection for one head pair (128 of the 256 v-dims);
                # pair 1's half runs inside the pair-1 loop as PE filler
                pt = mm_ps.tile([128, 512], F32, tag="mmps")
                c0 = QK + ph * 128
                for et in range(ET):
                    nc.tensor.matmul(
                        pt[:, :128],
                        xt_sb[:, et, st * 128 : (st + 1) * 128],
                        wt_sb[:, et, c0 : c0 + 128],
                        start=(et == 0),
                        stop=False,
                    )
                nc.tensor.matmul(  # + ones^T bv (bias row)
                    pt[:, :128],
                    ones_sb[0:1, 0:128],
                    bv_sb[0:1, ph * 128 : (ph + 1) * 128],
                    start=False,
                    stop=True,
                )
                nc.vector.tensor_copy(
                    v_sb[:, st, ph * 128 : (ph + 1) * 128], pt[:, :128]
                )

            def emit_d_group(p, st, out_dram):
                # in-loop variant: mm_ps tiles + DVE copies; output DMAs
                # alternate SP/gpsimd queues so neither backs up
                ot = foutp.tile([128, E], BF16, tag="fout", name=f"fo_{p}_{st}")
                for nck in range(2):
                    pt = mm_ps.tile([128, 512], F32, tag="mmps", name=f"fp_{p}_{st}_{nck}")
                    nc.tensor.matmul(
                        pt[:],
                        outT_bf[:, p, st * 128 : (st + 1) * 128],
                        wo_sb[:, p, nck * 512 : (nck + 1) * 512],
                        start=True,
                        stop=True,
                    )
                    nc.vector.tensor_copy(ot[:, nck * 512 : (nck + 1) * 512], pt[:])
                dq = nc.sync if st % 2 == 0 else nc.gpsimd
                dq.dma_start(out_dram[st * 128 : (st + 1) * 128, :], ot[:])

            def emit_d_group_sp(p, st, out_dram):
                # tail variant: freed score-PSUM tile, one copy, DVE/ACT alternating
                ot = foutp.tile([128, E], BF16, tag="fout", name=f"fo_{p}_{st}")
                pt = sp_ps.tile([128, 1024], F32, tag="sp", name=f"fsp_{p}_{st}")
                for nck in range(2):
                    nc.tensor.matmul(
                        pt[:, nck * 512 : (nck + 1) * 512],
                        outT_bf[:, p, st * 128 : (st + 1) * 128],
                        wo_sb[:, p, nck * 512 : (nck + 1) * 512],
                        start=True,
                        stop=True,
                    )
                if st % 2 == 0:
                    nc.vector.tensor_copy(ot[:], pt[:])
                else:
                    nc.scalar.copy(ot[:], pt[:])
                dq = nc.sync if st % 2 == 0 else nc.gpsimd
                dq.dma_start(out_dram[st * 128 : (st + 1) * 128, :], ot[:])

            # ---- attn.V slices: col-tiled hh pairs -----------------------
            c_state = {}

            def emit_c_slices(p, g, half, jpair, exs, vss):
                if jpair == 0:
                    c_state[half] = ot_ps.tile(
                        [128, 1024], F32, tag="otps", name=f"oTt_{p}_{g}_{half}"
                    )
                oTt = c_state[half]
                for j in (2 * jpair, 2 * jpair + 1):
                    kt = FG * g + j
                    for qc in range(2):
                        q0 = half * 1024 + qc * 512
                        for hh in range(2):
                            nc.tensor.matmul(
                                oTt[
                                    hh * 64 : (hh + 1) * 64,
                                    qc * 512 : (qc + 1) * 512,
                                ],
                                vss[kt][:, hh, :],
                                exs[kt][:, hh, q0 : q0 + 512],
                                start=(j == 0),
                                stop=(j == FG - 1),
                            )
                if jpair == 1:
                    dst = outT_f32[:, p, half * 1024 : (half + 1) * 1024]
                    if g == 0:
                        nc.vector.tensor_copy(dst, oTt[:])
                    else:
                        nc.vector.tensor_add(dst, dst, oTt[:])

            def emit_cast(p, half):
                nc.vector.tensor_copy(
                    outT_bf[:, p, half * 1024 : (half + 1) * 1024],
                    outT_f32[:, p, half * 1024 : (half + 1) * 1024],
                )

            # ---- pre-attention ------------------------------------------
            # warmup matmuls (K=1 on the memset ones row) ramp the PE clock
            # while the input DMAs stream in
            wpt = mm_ps.tile([128, 512], F32, tag="mmps", name="warm")
            for r in range(18):
                nc.tensor.matmul(
                    wpt[:],
                    ones_sb[0:1, 0:128],
                    ones_sb[0:1, 0:512],
                    start=True,
                    stop=True,
                )
            emit_qk_group(0, 0)  # Q pair0 cols 0-511
            emit_qk_group(0, 1)
            emit_qk_group(2, 0)  # K pair0 cols 0-511 (kts 0-3)

            def qg(eo, sc):
                return lambda: emit_qk_group(eo, sc)

            def vg(st, ph):
                return lambda: emit_v_group(st, ph)

            def dg(p, st, out_dram):
                return lambda: emit_d_group(p, st, out_dram)

            def cs(p, g, half, jpair, exs, vss):
                return lambda: emit_c_slices(p, g, half, jpair, exs, vss)

            def ct(p, half):
                return lambda: emit_cast(p, half)

            # ---- attention per head pair ---------------------------------
            # Each pair walks a (kt, half) schedule; afterwork[pos] is the
            # PE work emitted right after that position's score matmuls+exps
            # (the previous group's attn.V slices + projection fillers).
            # Pair 0 defers k-tiles 0-4's second q-half so the exp stream
            # starts on sc0/sc1 input data while sc2/sc3 still stream in.
            def CS(p, kc, exs, vss):
                return lambda: emit_c_slices(
                    p, kc // FG - 1, (kc % FG) // 2, kc % 2, exs, vss
                )

            p0_exs = None
            p0_vss = None
            for p in range(2):
                exs = {}
                vss = {}
                dens = {}
                if p == 0:
                    sched = [
                        (0, 0), (1, 0), (2, 0), (3, 0), (4, 0),
                        (0, 1), (1, 1), (2, 1), (3, 1), (4, 1),
                    ] + [(kt, h) for kt in range(5, KT) for h in (0, 1)]
                    aw = {
                        0: [vg(0, 0), vg(1, 0)],
                        1: [qg(2, 1), vg(2, 0)],
                        2: [vg(3, 0), qg(0, 2)],
                        3: [vg(4, 0), qg(0, 3)],
                        4: [qg(2, 2), vg(5, 0)],
                        5: [vg(6, 0)],
                        6: [qg(2, 3), vg(7, 0)],
                        7: [vg(8, 0)],
                        8: [vg(9, 0)],
                        9: [CS(0, 4, exs, vss), vg(10, 0)],
                        10: [CS(0, 5, exs, vss)],
                        11: [qg(1, 0), vg(11, 0)],
                        12: [CS(0, 6, exs, vss)],
                        13: [qg(1, 1), vg(12, 0)],
                        14: [CS(0, 7, exs, vss)],
                        15: [vg(13, 0)],
                        16: [CS(0, 8, exs, vss)],
                        17: [vg(14, 0)],
                        18: [CS(0, 9, exs, vss)],
                        19: [vg(15, 0)],
                        20: [CS(0, 10, exs, vss)],
                        21: [qg(3, 0), vg(8, 1)],
                        22: [CS(0, 11, exs, vss)],
                        23: [qg(3, 1), vg(9, 1)],
                        24: [CS(0, 12, exs, vss), vg(10, 1)],
                        25: [vg(11, 1)],
                        26: [CS(0, 13, exs, vss), vg(12, 1)],
                        27: [vg(13, 1)],
                        28: [CS(0, 14, exs, vss)],
                        29: [qg(1, 2)],
                        30: [CS(0, 15, exs, vss)],
                    }
                else:
                    sched = [(kt, h) for kt in range(KT) for h in (0, 1)]
                    aw = {
                        0: [qg(1, 3)],
                        1: [vg(0, 1), vg(1, 1)],
                        2: [cs(0, 3, 0, 0, p0_exs, p0_vss)],
                        3: [cs(0, 3, 0, 1, p0_exs, p0_vss), vg(2, 1)],
                        4: [ct(0, 0)],
                        5: [cs(0, 3, 1, 0, p0_exs, p0_vss), vg(3, 1)],
                        6: [cs(0, 3, 1, 1, p0_exs, p0_vss)],
                        7: [ct(0, 1), vg(4, 1)],
                        8: [CS(1, 4, exs, vss), dg(0, 0, out0)],
                        9: [dg(0, 1, out0), vg(5, 1)],
                        10: [CS(1, 5, exs, vss), qg(3, 2)],
                        11: [dg(0, 2, out0), vg(6, 1)],
                        12: [CS(1, 6, exs, vss), dg(0, 3, out0)],
                        13: [dg(0, 4, out0), vg(7, 1)],
                        14: [CS(1, 7, exs, vss), dg(0, 5, out0)],
                        15: [dg(0, 6, out0), dg(0, 7, out0)],
                        16: [CS(1, 8, exs, vss), qg(3, 3)],
                        17: [dg(0, 8, out0)],
                        18: [CS(1, 9, exs, vss), dg(0, 9, out0)],
                        19: [dg(0, 10, out0)],
                        20: [CS(1, 10, exs, vss), dg(0, 11, out0)],
                        21: [dg(0, 12, out0), dg(0, 13, out0)],
                        22: [CS(1, 11, exs, vss)],
                        23: [dg(0, 14, out0)],
                        24: [CS(1, 12, exs, vss), dg(0, 15, out0)],
                        26: [CS(1, 13, exs, vss)],
                        27: [vg(14, 1)],
                        28: [CS(1, 14, exs, vss)],
                        29: [vg(15, 1)],
                        30: [CS(1, 15, exs, vss), cs(1, 3, 0, 0, exs, vss)],
                    }
                for pos, (kt, half) in enumerate(sched):
                    if kt not in exs:
                        exs[kt] = expp.tile(
                            [128, 2, S], BF16, tag="exp", name=f"ex_{p}_{kt}"
                        )
                        dens[kt] = smalls.tile(
                            [128, 2, 2], F32, tag="den", name=f"den_{p}_{kt}"
                        )
                    ex = exs[kt]
                    den = dens[kt]
                    sps = {}
                    for hh in range(2):
                        sps[hh] = sp_ps.tile(
                            [128, 1024], F32, tag="sp", name=f"sp_{p}_{kt}_{half}_{hh}"
                        )
                    for qc in range(2):
                        q0 = half * 1024 + qc * 512
                        for hh in range(2):
                            nc.tensor.matmul(
                                sps[hh][:, qc * 512 : (qc + 1) * 512],
                                qk_sb[
                                    hh * 64 : (hh + 1) * 64,
                                    2 + p,
                                    kt * 128 : (kt + 1) * 128,
                                ],
                                qk_sb[hh * 64 : (hh + 1) * 64, p, q0 : q0 + 512],
                                start=True,
                                stop=True,
                            )
                    for hh in range(2):
                        nc.scalar.activation(
                            ex[:, hh, half * 1024 : (half + 1) * 1024],
                            sps[hh][:],
                            AF.Exp,
                            scale=0.125,
                            accum_out=den[:, hh, half : half + 1],
                        )
                    for w in aw.get(pos, []):
                        w()
                    if half == 1:
                        dsum = smalls.tile([128, 2], F32, tag="dsum")
                        nc.vector.tensor_add(dsum[:], den[:, :, 0], den[:, :, 1])
                        rec = smalls.tile([128, 2], F32, tag="rec")
                        nc.vector.reciprocal(rec[:], dsum[:])
                        vs = vsp.tile([128, 2, DH], BF16, tag="vs")
                        vss[kt] = vs
                        for hh in range(2):
                            nc.vector.tensor_scalar_mul(
                                vs[:, hh, :],
                                in0=v_sb[
                                    :, kt, (2 * p + hh) * 64 : (2 * p + hh + 1) * 64
                                ],
                                scalar1=rec[:, hh : hh + 1],
                            )
                if p == 0:
                    p0_exs = exs
                    p0_vss = vss

            # ---- tail: pair1 group3 attn.V + projection ------------------
            # half0's projection overlaps the final half1 exps
            emit_c_slices(1, 3, 0, 1, exs, vss)
            emit_cast(1, 0)
            for st in range(ST // 2):
                emit_d_group_sp(1, st, out1)
            emit_c_slices(1, 3, 1, 0, exs, vss)
            emit_c_slices(1, 3, 1, 1, exs, vss)
            emit_cast(1, 1)
            for st in range(ST // 2, ST):
                emit_d_group_sp(1, st, out1)

    nc.compile()
    return nc


def _shard_inputs(input, Wqkv, bqkv, Wo):
    """Build the 8 per-core input dicts (host-side layout/sharding)."""
    bf16 = ml_dtypes.bfloat16
    in_maps = []
    for c in range(NCORES):
        b = c // 4
        g = c % 4
        heads = range(4 * g, 4 * g + 4)
        rows = (
            [slice(64 * h, 64 * h + 64) for h in heads]
            + [slice(E + 64 * h, E + 64 * h + 64) for h in heads]
            + [slice(2 * E + 64 * h, 2 * E + 64 * h + 64) for h in heads]
        )
        W_sel = np.concatenate([Wqkv[s] for s in rows], axis=0)  # [768, 1024]
        b_sel = np.concatenate([bqkv[s] for s in rows], axis=0)  # [768]
        xf = input[b].T.reshape(ET, 128, S)  # [et, p, s]
        wf = W_sel.T.reshape(ET, 128, V3)  # [et, p, v]
        in_maps.append(
            {
                "xTa": np.ascontiguousarray(
                    xf[:, :, 0:1024].transpose(1, 0, 2).reshape(128, ET * 1024)
                ).astype(bf16),
                "xTb": np.ascontiguousarray(
                    xf[:, :, 1024:2048].transpose(1, 0, 2).reshape(128, ET * 1024)
                ).astype(bf16),
                "wTq": np.ascontiguousarray(
                    wf[:, :, 0:256].transpose(1, 0, 2).reshape(128, ET * 256)
                ).astype(bf16),
                "wTk": np.ascontiguousarray(
                    wf[:, :, 256:QK].transpose(1, 0, 2).reshape(128, ET * 256)
                ).astype(bf16),
                "wTv": np.ascontiguousarray(
                    wf[:, :, QK:V3].transpose(1, 0, 2).reshape(128, ET * 256)
                ).astype(bf16),
                "bq": np.ascontiguousarray(b_sel[:QK].reshape(4, 128).T),
                "bv": np.ascontiguousarray(b_sel[QK:V3].reshape(1, 256)).astype(bf16),
                "woT": np.ascontiguousarray(
                    Wo[:, 4 * g * DH : 4 * (g + 1) * DH].T
                ).astype(bf16),
            }
        )
    return in_maps


def kernel(input, Wqkv, bqkv, Wo, bo, _trace=False):
    global LAST_RESULTS
    input = np.asarray(input, dtype=np.float32)
    Wqkv = np.asarray(Wqkv, dtype=np.float32)
    bqkv = np.asarray(bqkv, dtype=np.float32)
    Wo = np.asarray(Wo, dtype=np.float32)
    bo = np.asarray(bo, dtype=np.float32)

    nc = build_kernel()
    in_maps = _shard_inputs(input, Wqkv, bqkv, Wo)
    kwargs = {}
    if _trace:
        kwargs = dict(trace=True, trace_cores=[0])
    res = run_bass_kernel_spmd(nc, in_maps, core_ids=list(range(NCORES)), **kwargs)
    LAST_RESULTS = res

    out = np.zeros((B, S, E), dtype=np.float32)
    for c in range(NCORES):
        out[c // 4] += res.results[c]["out0"].astype(np.float32)
        out[c // 4] += res.results[c]["out1"].astype(np.float32)
    out += bo
    return out


# revision 43
# speedup vs baseline: 1.2909x; 1.0067x over previous
"""Multi-head attention (softmax over the QUERY axis) on 8 TRN2 NeuronCores.

Sharding: 2 batches x 4 head-groups (4 heads each) -> 8 cores.
Each core computes, for its (batch b, heads 4g..4g+3):
    qkT = W_{q,k} @ x_b^T + b_{q,k}   [512, 2048]   (e_out on partitions)
    V   = x_b @ W_v^T + b_v           [2048, 256]
    S'  = K Q^T (scores TRANSPOSED)   [k, q] per head
    P   = exp(S'/8) with fused row-sum -> denom[k]  (softmax over q == free dim)
    outT= sum_k (V[k,:]/denom[k]) P[k,:]            [d, q] per head
    part= outT^T @ WoT_g              [2048, 1024]  (partial per head pair)
Host sums the partials per batch (bf16) and adds bo.

The ACT exp stream (~165us/core) is the critical path; everything else
is scheduled around keeping ACT saturated and the PE clock warm:
- attn.V matmuls (M=64) use disjoint PSUM partition quadrants per head
  (PE col tiling) and are emitted hh-adjacent so pairs co-execute.
- Per k-tile: [scores half0][exp x2][attn.V prev group + fillers]
  [scores half1][exp x2][fillers][den -> vs]. Fillers (qkT/V projection
  groups, pair0's tail, output projection groups) follow an explicit
  per-kt schedule that keeps PE gaps small in BOTH pair loops -- idle
  gaps >~2us drop the PE to its mid p-state (2x slower) which then
  delays score refills and starves ACT.
- Input DMA is spread over three engine queues (descriptor issue on one
  queue serializes at ~0.6us each).
- The tail projects q-half 0 while the final exps drain, using the
  freed score-PSUM tiles and copies alternating DVE/ACT.
"""

import sys

if "/opt/trn_rl_repo" not in sys.path:
    sys.path.insert(0, "/opt/trn_rl_repo")

import numpy as np
import ml_dtypes

import concourse.bass as bass
import concourse.mybir as mybir
import concourse.tile as tile
from concourse import bacc
from concourse.bass_utils import run_bass_kernel_spmd

F32 = mybir.dt.float32
BF16 = mybir.dt.bfloat16
AF = mybir.ActivationFunctionType

B, S, E, H = 2, 2048, 1024, 16
HL = 4  # heads per core
DH = 64
QK = 512  # q+k out dims per core (2*HL*DH)
V3 = 768  # q+k+v out dims per core
NCORES = 8

ET = E // 128  # 8 e-tiles
ST = S // 128  # 16 s-tiles
SC = S // 512  # 4 s/q chunks of 512
KT = ST  # 16 k-tiles
FG = 4  # k-tiles per attn.V accumulation group

LAST_RESULTS = None


def build_kernel():
    nc = bacc.Bacc("TRN2", target_bir_lowering=False, debug=False, num_devices=NCORES)

    # inputs are packed host-side in partition-major et-blocked layout so
    # each loads with ONE contiguous-per-partition DMA (issue cost dominates
    # DMA throughput: ~2us per issue regardless of size)
    xT0 = nc.dram_tensor("xT0", [128, ET * 512], BF16, kind="ExternalInput")
    xT1 = nc.dram_tensor("xT1", [128, ET * 512], BF16, kind="ExternalInput")
    xTb = nc.dram_tensor("xTb", [128, ET * 1024], BF16, kind="ExternalInput")
    wTq = nc.dram_tensor("wTq", [128, ET * 256], BF16, kind="ExternalInput")
    wTk = nc.dram_tensor("wTk", [128, ET * 256], BF16, kind="ExternalInput")
    wTv = nc.dram_tensor("wTv", [128, ET * 256], BF16, kind="ExternalInput")
    bq = nc.dram_tensor("bq", [128, 4], F32, kind="ExternalInput")
    bv = nc.dram_tensor("bv", [1, 256], BF16, kind="ExternalInput")
    woT = nc.dram_tensor("woT", [2 * 128, E], BF16, kind="ExternalInput")
    out0 = nc.dram_tensor("out0", [S, E], BF16, kind="ExternalOutput")
    out1 = nc.dram_tensor("out1", [S, E], BF16, kind="ExternalOutput")

    with tile.TileContext(nc) as tc:
        with (
            tc.tile_pool(name="persist", bufs=1) as persist,
            tc.tile_pool(name="smalls", bufs=6) as smalls,
            tc.tile_pool(name="expp", bufs=2 * FG + 1) as expp,
            tc.tile_pool(name="vsp", bufs=2 * FG + 2) as vsp,
            tc.tile_pool(name="fout", bufs=3) as foutp,
            tc.tile_pool(name="mm_ps", bufs=2, space="PSUM") as mm_ps,
            tc.tile_pool(name="sp_ps", bufs=2, space="PSUM") as sp_ps,
            tc.tile_pool(name="ot_ps", bufs=1, space="PSUM") as ot_ps,
        ):
            qk_sb = persist.tile([128, 4, S], BF16, tag="qk")
            v_sb = persist.tile([128, ST, 256], F32, tag="v")
            outT_f32 = persist.tile([128, 2, S], F32, tag="outT")
            outT_bf = persist.tile([128, 2, S], BF16, tag="outT_bf")
            bq_sb = persist.tile([128, 4], F32, tag="bq")
            bv_sb = persist.tile([1, 256], BF16, tag="bv")
            ones_sb = persist.tile([1, 512], BF16, tag="ones")
            xt_sb = persist.tile([128, ET, S], BF16, tag="xt")
            wt_sb = persist.tile([128, ET, V3], BF16, tag="wt")
            wo_sb = persist.tile([128, 2, E], BF16, tag="wo")

            nc.vector.memset(ones_sb[:], 1.0)
            # the ACT queue must carry NO dma issues (they would delay every
            # exp); big contiguous DMAs in strict priority order on one queue
            # (in-flow is bandwidth-bound; a second queue would steal engines
            # from the critical prefix)
            nc.sync.dma_start(wt_sb[:, :, 0:256], wTq[:])
            nc.sync.dma_start(xt_sb[:, :, 0:512], xT0[:])
            nc.sync.dma_start(wt_sb[:, :, 256:QK], wTk[:])
            nc.sync.dma_start(xt_sb[:, :, 512:1024], xT1[:])
            nc.sync.dma_start(wt_sb[:, :, QK:V3], wTv[:])
            nc.sync.dma_start(xt_sb[:, :, 1024:2048], xTb[:])
            nc.gpsimd.dma_start(bq_sb[:], bq[:])
            nc.gpsimd.dma_start(bv_sb[:], bv[:])
            for p in range(2):
                nc.gpsimd.dma_start(wo_sb[:, p, :], woT[p * 128 : (p + 1) * 128, :])

            # ---- emitters for qkT / V accumulation groups ----------------
            def emit_qk_group(eo, sc):
                pt = mm_ps.tile([128, 512], F32, tag="mmps")
                for et in range(ET):
                    nc.tensor.matmul(
                        pt[:],
                        wt_sb[:, et, eo * 128 : (eo + 1) * 128],
                        xt_sb[:, et, sc * 512 : (sc + 1) * 512],
                        start=(et == 0),
                        stop=(et == ET - 1),
                    )
                nc.vector.tensor_scalar_add(
                    qk_sb[:, eo, sc * 512 : (sc + 1) * 512],
                    in0=pt[:],
                    scalar1=bq_sb[:, eo : eo + 1],
                )

            def emit_v_group(st, ph):
                # V projection for one head pair (128 of the 256 v-dims);
                # pair 1's half runs inside the pair-1 loop as PE filler
                pt = mm_ps.tile([128, 512], F32, tag="mmps")
                c0 = QK + ph * 128
                for et in range(ET):
                    nc.tensor.matmul(
                        pt[:, :128],
                        xt_sb[:, et, st * 128 : (st + 1) * 128],
                        wt_sb[:, et, c0 : c0 + 128],
                        start=(et == 0),
                        stop=False,
                    )
                nc.tensor.matmul(  # + ones^T bv (bias row)
                    pt[:, :128],
                    ones_sb[0:1, 0:128],
                    bv_sb[0:1, ph * 128 : (ph + 1) * 128],
                    start=False,
                    stop=True,
                )
                nc.vector.tensor_copy(
                    v_sb[:, st, ph * 128 : (ph + 1) * 128], pt[:, :128]
                )

            def emit_d_group(p, st, out_dram):
                # in-loop variant: mm_ps tiles + DVE copies; output DMAs
                # alternate SP/gpsimd queues so neither backs up
                ot = foutp.tile([128, E], BF16, tag="fout", name=f"fo_{p}_{st}")
                for nck in range(2):
                    pt = mm_ps.tile([128, 512], F32, tag="mmps", name=f"fp_{p}_{st}_{nck}")
                    nc.tensor.matmul(
                        pt[:],
                        outT_bf[:, p, st * 128 : (st + 1) * 128],
                        wo_sb[:, p, nck * 512 : (nck + 1) * 512],
                        start=True,
                        stop=True,
                    )
                    nc.vector.tensor_copy(ot[:, nck * 512 : (nck + 1) * 512], pt[:])
                dq = nc.sync if st % 2 == 0 else nc.gpsimd
                dq.dma_start(out_dram[st * 128 : (st + 1) * 128, :], ot[:])

            def emit_d_group_sp(p, st, out_dram):
                # tail variant: freed score-PSUM tile, one copy, DVE/ACT alternating
                ot = foutp.tile([128, E], BF16, tag="fout", name=f"fo_{p}_{st}")
                pt = sp_ps.tile([128, 1024], F32, tag="sp", name=f"fsp_{p}_{st}")
                for nck in range(2):
                    nc.tensor.matmul(
                        pt[:, nck * 512 : (nck + 1) * 512],
                        outT_bf[:, p, st * 128 : (st + 1) * 128],
                        wo_sb[:, p, nck * 512 : (nck + 1) * 512],
                        start=True,
                        stop=True,
                    )
                if st % 2 == 0:
                    nc.vector.tensor_copy(ot[:], pt[:])
                else:
                    nc.scalar.copy(ot[:], pt[:])
                dq = nc.sync if st % 2 == 0 else nc.gpsimd
                dq.dma_start(out_dram[st * 128 : (st + 1) * 128, :], ot[:])

            # ---- attn.V slices: col-tiled hh pairs -----------------------
            c_state = {}

            def emit_c_slices(p, g, half, jpair, exs, vss):
                if jpair == 0:
                    c_state[half] = ot_ps.tile(
                        [128, 1024], F32, tag="otps", name=f"oTt_{p}_{g}_{half}"
                    )
                oTt = c_state[half]
                for j in (2 * jpair, 2 * jpair + 1):
                    kt = FG * g + j
                    for qc in range(2):
                        q0 = half * 1024 + qc * 512
                        for hh in range(2):
                            nc.tensor.matmul(
                                oTt[
                                    hh * 64 : (hh + 1) * 64,
                                    qc * 512 : (qc + 1) * 512,
                                ],
                                vss[kt][:, hh, :],
                                exs[kt][:, hh, q0 : q0 + 512],
                                start=(j == 0),
                                stop=(j == FG - 1),
                            )
                if jpair == 1:
                    dst = outT_f32[:, p, half * 1024 : (half + 1) * 1024]
                    if g == 0:
                        nc.vector.tensor_copy(dst, oTt[:])
                    else:
                        nc.vector.tensor_add(dst, dst, oTt[:])

            def emit_cast(p, half):
                nc.vector.tensor_copy(
                    outT_bf[:, p, half * 1024 : (half + 1) * 1024],
                    outT_f32[:, p, half * 1024 : (half + 1) * 1024],
                )

            # ---- pre-attention ------------------------------------------
            # warmup matmuls (K=1 on the memset ones row) ramp the PE clock
            # while the input DMAs stream in
            wpt = mm_ps.tile([128, 512], F32, tag="mmps", name="warm")
            for r in range(18):
                nc.tensor.matmul(
                    wpt[:],
                    ones_sb[0:1, 0:128],
                    ones_sb[0:1, 0:512],
                    start=True,
                    stop=True,
                )
            emit_qk_group(0, 0)  # Q pair0 cols 0-511  (needs wTq + xT0)
            emit_qk_group(2, 0)  # K pair0 cols 0-511  (needs wTk + xT0)
            emit_qk_group(0, 1)  # Q pair0 cols 512-1023 (needs xT1)

            def qg(eo, sc):
                return lambda: emit_qk_group(eo, sc)

            def vg(st, ph):
                return lambda: emit_v_group(st, ph)

            def dg(p, st, out_dram):
                return lambda: emit_d_group(p, st, out_dram)

            def cs(p, g, half, jpair, exs, vss):
                return lambda: emit_c_slices(p, g, half, jpair, exs, vss)

            def ct(p, half):
                return lambda: emit_cast(p, half)

            # ---- attention per head pair ---------------------------------
            # Each pair walks a (kt, half) schedule; afterwork[pos] is the
            # PE work emitted right after that position's score matmuls+exps
            # (the previous group's attn.V slices + projection fillers).
            # Pair 0 defers k-tiles 0-4's second q-half so the exp stream
            # starts on sc0/sc1 input data while sc2/sc3 still stream in.
            def CS(p, kc, exs, vss):
                return lambda: emit_c_slices(
                    p, kc // FG - 1, (kc % FG) // 2, kc % 2, exs, vss
                )

            p0_exs = None
            p0_vss = None
            for p in range(2):
                exs = {}
                vss = {}
                dens = {}
                if p == 0:
                    sched = [
                        (0, 0), (1, 0), (2, 0), (3, 0), (4, 0),
                        (0, 1), (1, 1), (2, 1), (3, 1), (4, 1),
                    ] + [(kt, h) for kt in range(5, KT) for h in (0, 1)]
                    aw = {
                        0: [vg(0, 0), vg(1, 0)],
                        1: [qg(2, 1), vg(2, 0)],
                        2: [vg(3, 0), qg(0, 2)],
                        3: [vg(4, 0), qg(0, 3)],
                        4: [qg(2, 2), vg(5, 0)],
                        5: [vg(6, 0)],
                        6: [qg(2, 3), vg(7, 0)],
                        7: [vg(8, 0)],
                        8: [vg(9, 0)],
                        9: [CS(0, 4, exs, vss), vg(10, 0)],
                        10: [CS(0, 5, exs, vss)],
                        11: [qg(1, 0), vg(11, 0)],
                        12: [CS(0, 6, exs, vss)],
                        13: [qg(1, 1), vg(12, 0)],
                        14: [CS(0, 7, exs, vss)],
                        15: [vg(13, 0)],
                        16: [CS(0, 8, exs, vss)],
                        17: [vg(14, 0)],
                        18: [CS(0, 9, exs, vss)],
                        19: [vg(15, 0)],
                        20: [CS(0, 10, exs, vss)],
                        21: [qg(3, 0), vg(8, 1)],
                        22: [CS(0, 11, exs, vss)],
                        23: [qg(3, 1), vg(9, 1)],
                        24: [CS(0, 12, exs, vss), vg(10, 1)],
                        25: [vg(11, 1)],
                        26: [CS(0, 13, exs, vss), vg(12, 1)],
                        27: [vg(13, 1)],
                        28: [CS(0, 14, exs, vss)],
                        29: [qg(1, 2)],
                        30: [CS(0, 15, exs, vss)],
                    }
                else:
                    sched = [(kt, h) for kt in range(KT) for h in (0, 1)]
                    aw = {
                        0: [qg(1, 3)],
                        1: [vg(0, 1), vg(1, 1)],
                        2: [cs(0, 3, 0, 0, p0_exs, p0_vss)],
                        3: [cs(0, 3, 0, 1, p0_exs, p0_vss), vg(2, 1)],
                        4: [ct(0, 0)],
                        5: [cs(0, 3, 1, 0, p0_exs, p0_vss), vg(3, 1)],
                        6: [cs(0, 3, 1, 1, p0_exs, p0_vss)],
                        7: [ct(0, 1), vg(4, 1)],
                        8: [CS(1, 4, exs, vss), dg(0, 0, out0)],
                        9: [dg(0, 1, out0), vg(5, 1)],
                        10: [CS(1, 5, exs, vss), qg(3, 2)],
                        11: [dg(0, 2, out0), vg(6, 1)],
                        12: [CS(1, 6, exs, vss), dg(0, 3, out0)],
                        13: [dg(0, 4, out0), vg(7, 1)],
                        14: [CS(1, 7, exs, vss), dg(0, 5, out0)],
                        15: [dg(0, 6, out0), dg(0, 7, out0)],
                        16: [CS(1, 8, exs, vss), qg(3, 3)],
                        17: [dg(0, 8, out0)],
                        18: [CS(1, 9, exs, vss), dg(0, 9, out0)],
                        19: [dg(0, 10, out0)],
                        20: [CS(1, 10, exs, vss), dg(0, 11, out0)],
                        21: [dg(0, 12, out0), dg(0, 13, out0)],
                        22: [CS(1, 11, exs, vss)],
                        23: [dg(0, 14, out0)],
                        24: [CS(1, 12, exs, vss), dg(0, 15, out0)],
                        26: [CS(1, 13, exs, vss)],
                        27: [vg(14, 1)],
                        28: [CS(1, 14, exs, vss)],
                        29: [vg(15, 1)],
                        30: [CS(1, 15, exs, vss), cs(1, 3, 0, 0, exs, vss)],
                    }
                for pos, (kt, half) in enumerate(sched):
                    if kt not in exs:
                        exs[kt] = expp.tile(
                            [128, 2, S], BF16, tag="exp", name=f"ex_{p}_{kt}"
                        )
                        dens[kt] = smalls.tile(
                            [128, 2, 2], F32, tag="den", name=f"den_{p}_{kt}"
                        )
                    ex = exs[kt]
                    den = dens[kt]
                    sps = {}
                    for hh in range(2):
                        sps[hh] = sp_ps.tile(
                            [128, 1024], F32, tag="sp", name=f"sp_{p}_{kt}_{half}_{hh}"
                        )
                    for qc in range(2):
                        q0 = half * 1024 + qc * 512
                        for hh in range(2):
                            nc.tensor.matmul(
                                sps[hh][:, qc * 512 : (qc + 1) * 512],
                                qk_sb[
                                    hh * 64 : (hh + 1) * 64,
                                    2 + p,
                                    kt * 128 : (kt + 1) * 128,
                                ],
                                qk_sb[hh * 64 : (hh + 1) * 64, p, q0 : q0 + 512],
                                start=True,
                                stop=True,
                            )
                    for hh in range(2):
                        nc.scalar.activation(
                            ex[:, hh, half * 1024 : (half + 1) * 1024],
                            sps[hh][:],
                            AF.Exp,
                            scale=0.125,
                            accum_out=den[:, hh, half : half + 1],
                        )
                    for w in aw.get(pos, []):
                        w()
                    if half == 1:
                        dsum = smalls.tile([128, 2], F32, tag="dsum")
                        nc.vector.tensor_add(dsum[:], den[:, :, 0], den[:, :, 1])
                        rec = smalls.tile([128, 2], F32, tag="rec")
                        nc.vector.reciprocal(rec[:], dsum[:])
                        vs = vsp.tile([128, 2, DH], BF16, tag="vs")
                        vss[kt] = vs
                        for hh in range(2):
                            nc.vector.tensor_scalar_mul(
                                vs[:, hh, :],
                                in0=v_sb[
                                    :, kt, (2 * p + hh) * 64 : (2 * p + hh + 1) * 64
                                ],
                                scalar1=rec[:, hh : hh + 1],
                            )
                if p == 0:
                    p0_exs = exs
                    p0_vss = vss

            # ---- tail: pair1 group3 attn.V + projection ------------------
            # half0's projection overlaps the final half1 exps
            emit_c_slices(1, 3, 0, 1, exs, vss)
            emit_cast(1, 0)
            for st in range(ST // 2):
                emit_d_group_sp(1, st, out1)
            emit_c_slices(1, 3, 1, 0, exs, vss)
            emit_c_slices(1, 3, 1, 1, exs, vss)
            emit_cast(1, 1)
            for st in range(ST // 2, ST):
                emit_d_group_sp(1, st, out1)

    nc.compile()
    return nc


def _shard_inputs(input, Wqkv, bqkv, Wo):
    """Build the 8 per-core input dicts (host-side layout/sharding)."""
    bf16 = ml_dtypes.bfloat16
    in_maps = []
    for c in range(NCORES):
        b = c // 4
        g = c % 4
        heads = range(4 * g, 4 * g + 4)
        rows = (
            [slice(64 * h, 64 * h + 64) for h in heads]
            + [slice(E + 64 * h, E + 64 * h + 64) for h in heads]
            + [slice(2 * E + 64 * h, 2 * E + 64 * h + 64) for h in heads]
        )
        W_sel = np.concatenate([Wqkv[s] for s in rows], axis=0)  # [768, 1024]
        b_sel = np.concatenate([bqkv[s] for s in rows], axis=0)  # [768]
        xf = input[b].T.reshape(ET, 128, S)  # [et, p, s]
        wf = W_sel.T.reshape(ET, 128, V3)  # [et, p, v]
        in_maps.append(
            {
                "xT0": np.ascontiguousarray(
                    xf[:, :, 0:512].transpose(1, 0, 2).reshape(128, ET * 512)
                ).astype(bf16),
                "xT1": np.ascontiguousarray(
                    xf[:, :, 512:1024].transpose(1, 0, 2).reshape(128, ET * 512)
                ).astype(bf16),
                "xTb": np.ascontiguousarray(
                    xf[:, :, 1024:2048].transpose(1, 0, 2).reshape(128, ET * 1024)
                ).astype(bf16),
                "wTq": np.ascontiguousarray(
                    wf[:, :, 0:256].transpose(1, 0, 2).reshape(128, ET * 256)
                ).astype(bf16),
                "wTk": np.ascontiguousarray(
                    wf[:, :, 256:QK].transpose(1, 0, 2).reshape(128, ET * 256)
                ).astype(bf16),
                "wTv": np.ascontiguousarray(
                    wf[:, :, QK:V3].transpose(1, 0, 2).reshape(128, ET * 256)
                ).astype(bf16),
                "bq": np.ascontiguousarray(b_sel[:QK].reshape(4, 128).T),
                "bv": np.ascontiguousarray(b_sel[QK:V3].reshape(1, 256)).astype(bf16),
                "woT": np.ascontiguousarray(
                    Wo[:, 4 * g * DH : 4 * (g + 1) * DH].T
                ).astype(bf16),
            }
        )
    return in_maps


def kernel(input, Wqkv, bqkv, Wo, bo, _trace=False):
    global LAST_RESULTS
    input = np.asarray(input, dtype=np.float32)
    Wqkv = np.asarray(Wqkv, dtype=np.float32)
    bqkv = np.asarray(bqkv, dtype=np.float32)
    Wo = np.asarray(Wo, dtype=np.float32)
    bo = np.asarray(bo, dtype=np.float32)

    nc = build_kernel()
    in_maps = _shard_inputs(input, Wqkv, bqkv, Wo)
    kwargs = {}
    if _trace:
        kwargs = dict(trace=True, trace_cores=[0])
    res = run_bass_kernel_spmd(nc, in_maps, core_ids=list(range(NCORES)), **kwargs)
    LAST_RESULTS = res

    out = np.zeros((B, S, E), dtype=np.float32)
    for c in range(NCORES):
        out[c // 4] += res.results[c]["out0"].astype(np.float32)
        out[c // 4] += res.results[c]["out1"].astype(np.float32)
    out += bo
    return out
